# revision 1
# baseline (speedup 1.0000x reference)
"""Trainium2 Bass kernel for KernelAttention (gaussian-kernel multi-head attention).

Math (per batch b):
  d2[q,k]   = |q_pos[q] - k_pos[k]|^2   (computed as -d2 via one K=5 augmented matmul)
  s_h[k,q]  = exp(-c_h * d2),  c_h = 1/lengthscale_h^2   (masked keys contribute 0)
  att_h[q,v]= sum_k s_h[k,q] * V[k,h,v] / (sum_k s_h[k,q]*unmasked[k] + 1e-5)
  out[q,o]  = sum_{h,v} att_h[q,v] * w_out[o, h*64+v]

Sharding: 8 cores = (batch b in 0..3) x (query half in 0..1); each core owns
[1024 q, 2048 k]. All inputs host-prepped per core; outputs gathered on host.

Device-side layout is score-transposed: s_h is [k, q] so the attend matmul
(lhsT = values, rhs = scores) contracts k on the PE partition dim directly.
Masking + normalization are folded in: masked V rows are zeroed on the host and
a per-head ones-column (also mask-zeroed) produces the normalizer as psum row 64.
Normalization is deferred past the attend; the reciprocal is computed via
ACT Ln/Exp and broadcast across partitions with a tiny K=8 selection matmul.
Two heads (c=4, c=0.04) are derived from their 4x-smaller neighbors by two DVE
squarings, offloading exp work from the scalar engine.
"""

import numpy as np
from contextlib import ExitStack

B, LQ, LK, DPOS = 4, 2048, 2048, 3
H, V, OUTD = 8, 64, 512
QS = LQ // 2          # q rows per core
KT = LK // 128        # k tiles
V1 = V + 1            # value cols + ones col
NCORES = 8

# head processing order: chain sources immediately before their derived heads
ORDER = [3, 2, 6, 5, 0, 1, 4, 7]
DERIVED = {2: 3, 5: 6}  # derived_head -> source_head, s_d = s_src ** 4

_cache = {}


def _build(cv, use_chains):
    key = (tuple(cv), use_chains)
    if key in _cache:
        return _cache[key]
    import concourse.bacc as bacc
    import concourse.tile as tile
    from concourse import mybir

    f32 = mybir.dt.float32
    bf16 = mybir.dt.bfloat16
    AF = mybir.ActivationFunctionType

    nc = bacc.Bacc("TRN2", target_bir_lowering=False, debug=False,
                   num_devices=NCORES)
    # ka/qa carry a hi/lo bf16 split of the K=5 augmented distance operands:
    # rows [hi(5); lo(5); hi(5)] x [hi(5); hi(5); lo(5)] so the single bf16
    # matmul accumulates hi*hi + lo*hi + hi*lo in fp32 PSUM (lo*lo ~2^-16 is
    # dropped). This avoids fp32 LOW_HIGH double-pass matmuls entirely.
    ka = nc.dram_tensor("ka", [15, LK], bf16, kind="ExternalInput").ap()
    qa = nc.dram_tensor("qa", [15, QS], bf16, kind="ExternalInput").ap()
    vp = nc.dram_tensor("vp", [128, KT, H * V1], bf16, kind="ExternalInput").ap()
    wt = nc.dram_tensor("wt", [128, 4, OUTD], bf16, kind="ExternalInput").ap()
    sel8 = nc.dram_tensor("sel8", [8, 4, 128], bf16, kind="ExternalInput").ap()
    outT = nc.dram_tensor("outT", [OUTD, QS], f32, kind="ExternalOutput").ap()

    with tile.TileContext(nc) as tc, ExitStack() as ctx:
        const = ctx.enter_context(tc.tile_pool(name="const", bufs=1))
        spool = ctx.enter_context(tc.tile_pool(name="spool", bufs=10))
        stage = ctx.enter_context(tc.tile_pool(name="stage", bufs=2))
        obuf = ctx.enter_context(tc.tile_pool(name="obuf", bufs=2))
        psp = ctx.enter_context(tc.tile_pool(name="psum", bufs=4, space="PSUM"))

        ka_sb = const.tile([15, LK], bf16)
        nc.sync.dma_start(out=ka_sb[:], in_=ka)
        qa_sb = const.tile([15, QS], bf16)
        nc.sync.dma_start(out=qa_sb[:], in_=qa)
        vp_sb = const.tile([128, KT, H * V1], bf16)
        nc.sync.dma_start(out=vp_sb[:], in_=vp)
        wt_sb = const.tile([128, 4, OUTD], bf16)
        nc.sync.dma_start(out=wt_sb[:], in_=wt)
        sel8_sb = const.tile([8, 4, 128], bf16)
        nc.sync.dma_start(out=sel8_sb[:], in_=sel8)

        # Phase A: m = -d2 in [k, q] layout, evacuated to bf16 SBUF.
        # m is split into NG group tiles (4 k-tiles each) so per-head exp /
        # squaring / attend pipeline at ~3.7us granularity — PE never idles
        # longer than the HAM MID window, staying at full clock.
        NG, GK = 4, KT // 4
        m_g = [const.tile([128, GK, QS], bf16, tag=f"m{g}", name=f"m{g}")
               for g in range(NG)]
        for kt in range(KT):
            d2 = psp.tile([128, QS], f32, tag="ps")
            for qc in range(2):
                s5 = slice(qc * 512, (qc + 1) * 512)
                nc.tensor.matmul(d2[:, s5],
                                 lhsT=ka_sb[:, kt * 128:(kt + 1) * 128],
                                 rhs=qa_sb[:, s5], start=True, stop=True)
            nc.vector.tensor_copy(out=m_g[kt // GK][:, kt % GK, :], in_=d2[:])

        flat = [const.tile([128, QS], bf16, tag=f"flat{j}", name=f"flat{j}")
                for j in range(4)]
        norms = const.tile([8, QS], f32)
        nc.vector.memset(norms[:], 1.0)
        eps_t = const.tile([8, 1], f32)
        nc.vector.memset(eps_t[:], 1e-5)
        lnn = const.tile([8, QS], f32)
        r_all = const.tile([8, QS], f32)
        r_hi = const.tile([8, QS], bf16)
        nc.vector.memset(r_hi[:], 0.0)
        r_lo = const.tile([8, QS], bf16)
        nc.vector.memset(r_lo[:], 0.0)

        s_tiles = {}
        for h in ORDER:
            sg = []
            att = psp.tile([V1, QS], f32, tag="ps")
            for g in range(NG):
                s = spool.tile([128, GK, QS], bf16, tag="s", name=f"s{h}_{g}")
                if use_chains and h in DERIVED:
                    src = s_tiles[DERIVED[h]][g]
                    nc.vector.tensor_mul(s[:], src[:], src[:])
                    nc.vector.tensor_mul(s[:], s[:], s[:])
                else:
                    nc.scalar.activation(out=s[:], in_=m_g[g][:], func=AF.Exp,
                                         scale=float(cv[h]))
                sg.append(s)
                for qc in range(2):
                    s5 = slice(qc * 512, (qc + 1) * 512)
                    for k2 in range(GK):
                        kt = g * GK + k2
                        nc.tensor.matmul(att[:, s5],
                                         lhsT=vp_sb[:, kt, h * V1:(h + 1) * V1],
                                         rhs=s[:, k2, s5],
                                         start=(kt == 0), stop=(kt == KT - 1))
            s_tiles[h] = sg
            r0 = (h % 2) * 64
            nc.vector.tensor_copy(out=flat[h // 2][r0:r0 + 64, :],
                                  in_=att[0:64, :])
            stg = stage.tile([V1, QS], f32, tag="stg")
            nc.vector.tensor_copy(out=stg[64:65, :], in_=att[64:65, :])
            nc.sync.dma_start(out=norms[h:h + 1, :], in_=stg[64:65, :])

        # r = 1/(norm + 1e-5) via exp(-ln(x)); Ln+Exp share one ACT table set
        nc.scalar.activation(out=lnn[:], in_=norms[:], func=AF.Ln, bias=eps_t[:])
        nc.scalar.activation(out=r_all[:], in_=lnn[:], func=AF.Exp, scale=-1.0)
        nc.vector.tensor_copy(out=r_hi[:], in_=r_all[:])
        nc.vector.tensor_sub(r_lo[:], r_all[:], r_hi[:])
        # broadcast r across partitions (head pair j -> 128 rows) and normalize
        for j in range(4):
            rb = psp.tile([128, QS], f32, tag="ps", name=f"rb{j}")
            for qc in range(2):
                s5 = slice(qc * 512, (qc + 1) * 512)
                nc.tensor.matmul(rb[:, s5], lhsT=sel8_sb[:, j, :],
                                 rhs=r_hi[:, s5], start=True, stop=False)
                nc.tensor.matmul(rb[:, s5], lhsT=sel8_sb[:, j, :],
                                 rhs=r_lo[:, s5], start=False, stop=True)
            nc.vector.tensor_mul(flat[j][:], flat[j][:], rb[:])

        # out projection: outT[o, q] = sum_hv wt[hv, o] * flat[hv, q]
        for ot in range(4):
            po = psp.tile([128, QS], f32, tag="ps", name=f"po{ot}")
            for j in range(4):
                for qc in range(2):
                    s5 = slice(qc * 512, (qc + 1) * 512)
                    nc.tensor.matmul(po[:, s5],
                                     lhsT=wt_sb[:, j, ot * 128:(ot + 1) * 128],
                                     rhs=flat[j][:, s5],
                                     start=(j == 0), stop=(j == 3))
            ob = obuf.tile([128, QS], f32, tag="ob", name=f"ob{ot}")
            if ot % 2 == 0:
                nc.scalar.copy(out=ob[:], in_=po[:])
            else:
                nc.vector.tensor_copy(out=ob[:], in_=po[:])
            nc.sync.dma_start(out=outT[ot * 128:(ot + 1) * 128, :], in_=ob[:])

    nc.compile()
    _cache[key] = nc
    return nc


def _hilo(x, bf16):
    hi = x.astype(bf16)
    lo = (x - hi.astype(np.float32)).astype(bf16)
    return hi, lo


def _prep_core(qp, kp, vals, mask, w_out, bf16):
    q2 = (qp * qp).sum(-1)
    one_q = np.ones(QS, np.float32)
    qa5 = np.stack([2 * qp[:, 0], 2 * qp[:, 1], 2 * qp[:, 2], -one_q, -q2]) \
        .astype(np.float32)
    k2 = (kp * kp).sum(-1)
    one_k = np.ones(LK, np.float32)
    ka5 = np.stack([kp[:, 0], kp[:, 1], kp[:, 2], k2, one_k]).astype(np.float32)
    ka_hi, ka_lo = _hilo(ka5, bf16)
    qa_hi, qa_lo = _hilo(qa5, bf16)
    ka = np.concatenate([ka_hi, ka_lo, ka_hi])   # [15, LK]
    qa = np.concatenate([qa_hi, qa_hi, qa_lo])   # [15, QS]
    vv = np.concatenate([vals, np.ones((LK, H, 1), np.float32)], axis=-1)
    vv = vv.copy()
    vv[mask] = 0.0
    vp = vv.reshape(KT, 128, H * V1).transpose(1, 0, 2).astype(bf16)
    wt = np.ascontiguousarray(w_out.T).reshape(4, 128, OUTD) \
        .transpose(1, 0, 2).astype(bf16)
    sel8 = np.zeros((8, 4, 128), np.float32)
    for j in range(4):
        sel8[2 * j, j, :64] = 1.0
        sel8[2 * j + 1, j, 64:] = 1.0
    return {"ka": np.ascontiguousarray(ka), "qa": np.ascontiguousarray(qa),
            "vp": np.ascontiguousarray(vp), "wt": np.ascontiguousarray(wt),
            "sel8": sel8.astype(bf16)}


def kernel(query_positions, key_positions, values, masked_elements,
           lengthscales, w_out, _want_trace=False):
    import ml_dtypes
    from concourse.bass_utils import run_bass_kernel_spmd

    bf16 = ml_dtypes.bfloat16
    qp = np.asarray(query_positions, np.float32)
    kp = np.asarray(key_positions, np.float32)
    vals = np.asarray(values, np.float32)
    mask = np.asarray(masked_elements).astype(bool)
    ls = np.asarray(lengthscales, np.float32)
    w = np.asarray(w_out, np.float32)

    cv = (1.0 / (ls.astype(np.float64) ** 2)).astype(np.float32)
    use_chains = all(
        np.float32(cv[d]) == np.float32(4.0) * np.float32(cv[s])
        for d, s in DERIVED.items())
    nc = _build(tuple(float(x) for x in cv), use_chains)

    in_maps = []
    for c in range(NCORES):
        b, hf = c // 2, c % 2
        in_maps.append(_prep_core(qp[b, hf * QS:(hf + 1) * QS], kp[b],
                                  vals[b], mask[b], w, bf16))
    res = run_bass_kernel_spmd(nc, in_maps, core_ids=list(range(NCORES)),
                               trace=_want_trace)
    out = np.empty((B, LQ, OUTD), np.float32)
    for c in range(NCORES):
        b, hf = c // 2, c % 2
        out[b, hf * QS:(hf + 1) * QS, :] = res.results[c]["outT"].T
    if _want_trace:
        return out, res
    return out



# revision 17
# speedup vs baseline: 1.1286x; 1.1286x over previous
"""Trainium2 Bass kernel for KernelAttention (gaussian-kernel multi-head attention).

Math (per batch b):
  d2[q,k]   = |q_pos[q] - k_pos[k]|^2   (as -d2 via one K=15 augmented matmul)
  s_h[k,q]  = exp(-c_h * d2),  c_h = 1/lengthscale_h^2
  att_h[q,v]= sum_k s_h[k,q] * V[k,h,v] / (sum_k s_h[k,q]*unmasked[k] + 1e-5)
  out[q,o]  = sum_{h,v} att_h[q,v] * w_out[o, h*64+v]

Sharding: 8 cores = (batch b in 0..3) x (query half in 0..1); each core owns
[1024 q, all keys].  All inputs host-prepped per core; outputs host-gathered.

Key structure (vs. a dense implementation):
- Masked keys are compacted away on the host (~half the keys), shrinking the
  k extent from 16 tiles to KT2 (9 here) tiles.  Padded tail keys carry
  zeroed values/ones-column so they contribute nothing.
- Scores stay transposed [k, q] so the attend matmul (lhsT = values+ones col,
  rhs = scores) contracts k on the PE partition dim; psum row 64 accumulates
  the normalizer via the mask-zeroed ones column.
- m = -d2 is evacuated from PSUM to fp32 SBUF (DVE/Pool alternating), so
  every exp reads full-precision distances (the bf16-m rounding amplified by
  c_h=100 was the baseline's dominant error).
- Large-lengthscale heads (c_h*d2max small) are factored through a rank-70
  polynomial basis: s_h ~= U[q,:] @ W_h[k,:]^T with monomial features of the
  augmented position vectors (host-computed).  Their attend collapses to
  G_h = W_h^T V_h (tiny) and att_h = G_h^T @ U^T, skipping both the exp and
  the O(k*q) attend matmuls.
- Normalization is deferred past the attend; reciprocals on DVE per head
  pair, broadcast across partitions with a K=4 hi/lo selection matmul.
"""

import numpy as np
import itertools
from math import factorial
from contextlib import ExitStack

B, LQ, LK, DPOS = 4, 2048, 2048, 3
H, V, OUTD = 8, 64, 512
QS = LQ // 2          # q rows per core
V1 = V + 1            # value cols + ones col
NCORES = 8
DMAX = 64.0           # poly fit domain [0, DMAX] for d2
PMAX = 4
FEATS = [f for f in itertools.product(range(PMAX + 1), repeat=4)
         if sum(f) <= PMAX]
R = len(FEATS)        # 70 poly features

_cache = {}


def _cheb_power_coeffs(c, deg, tmax):
    """exp(-c*t) ~= sum_j bt[j] t^j on [0, tmax]; returns (bt, max_err)."""
    from numpy.polynomial import chebyshev as C, polynomial as P
    t = (np.cos(np.pi * (np.arange(4000) + 0.5) / 4000) + 1) / 2 * tmax
    f = np.exp(-np.float64(c) * t)
    ch = C.chebfit(t / tmax * 2 - 1, f, deg)
    bpow = C.cheb2poly(ch)
    bt = np.zeros(deg + 1)
    acc = np.array([1.0])
    lin = np.array([-1.0, 2.0 / tmax])
    for j in range(deg + 1):
        bt[:len(acc)] += bpow[j] * acc
        acc = P.polymul(acc, lin)
    err = np.abs(np.polyval(bt[::-1], t) - f).max()
    return bt, err


def _classify(cv):
    """Split heads into poly-factorizable (large lengthscale) and exp heads."""
    poly = {}
    for h in range(H):
        for deg in range(2, PMAX + 1):
            bt, err = _cheb_power_coeffs(cv[h], deg, DMAX)
            if err < 2e-4:
                poly[h] = (deg, bt)
                break
    exp_heads = [h for h in range(H) if h not in poly]
    return poly, exp_heads


def _build(cv, KT2):
    key = (tuple(cv), KT2)
    if key in _cache:
        return _cache[key]
    import concourse.bacc as bacc
    import concourse.tile as tile
    from concourse import mybir

    f32 = mybir.dt.float32
    bf16 = mybir.dt.bfloat16
    AF = mybir.ActivationFunctionType

    poly, exp_heads = _classify(cv)
    n_poly = len(poly)
    poly_list = sorted(poly)
    NK = KT2 * 128

    nc = bacc.Bacc("TRN2", target_bir_lowering=False, debug=False,
                   num_devices=NCORES)
    # ka/qa: hi/lo bf16 split of the K=5 augmented distance operands:
    # rows [hi(5); lo(5); hi(5)] x [hi(5); hi(5); lo(5)] accumulate
    # hi*hi + lo*hi + hi*lo in fp32 PSUM in one K=15 matmul.
    ka = nc.dram_tensor("ka", [15, NK], bf16, kind="ExternalInput").ap()
    qa = nc.dram_tensor("qa", [15, QS], bf16, kind="ExternalInput").ap()
    vp = nc.dram_tensor("vp", [128, KT2, H * V1], bf16, kind="ExternalInput").ap()
    wt = nc.dram_tensor("wt", [128, 4, OUTD], bf16, kind="ExternalInput").ap()
    sel2 = nc.dram_tensor("sel2", [2, 128], bf16, kind="ExternalInput").ap()
    if n_poly:
        ut = nc.dram_tensor("ut", [R, QS], bf16, kind="ExternalInput").ap()
        wp = nc.dram_tensor("wp", [128, KT2, n_poly * R], bf16,
                            kind="ExternalInput").ap()
    outT = nc.dram_tensor("outT", [OUTD, QS], f32, kind="ExternalOutput").ap()

    with tile.TileContext(nc) as tc, ExitStack() as ctx:
        const = ctx.enter_context(tc.tile_pool(name="const", bufs=1))
        # 4 bufs for 5 score tensors: the 5th head's exp stream starts well
        # after the 1st head's attend has been evacuated, so it reuses buf 0
        spool = ctx.enter_context(tc.tile_pool(name="spool", bufs=4))
        gpool = ctx.enter_context(tc.tile_pool(name="gpool", bufs=2))
        stage = ctx.enter_context(tc.tile_pool(name="stage", bufs=2))
        rpool = ctx.enter_context(tc.tile_pool(name="rpool", bufs=2))
        obuf = ctx.enter_context(tc.tile_pool(name="obuf", bufs=2))
        psp = ctx.enter_context(tc.tile_pool(name="psum", bufs=4, space="PSUM"))

        ka_sb = const.tile([15, NK], bf16)
        nc.sync.dma_start(out=ka_sb[:], in_=ka)
        qa_sb = const.tile([15, QS], bf16)
        nc.sync.dma_start(out=qa_sb[:], in_=qa)
        vp_sb = const.tile([128, KT2, H * V1], bf16)
        nc.sync.dma_start(out=vp_sb[:], in_=vp)
        if n_poly:
            wp_sb = const.tile([128, KT2, n_poly * R], bf16)
            nc.sync.dma_start(out=wp_sb[:], in_=wp)
            ut_sb = const.tile([R, QS], bf16)
            nc.sync.dma_start(out=ut_sb[:], in_=ut)
        wt_sb = const.tile([128, 4, OUTD], bf16)
        nc.sync.dma_start(out=wt_sb[:], in_=wt)
        sel2_sb = const.tile([2, 128], bf16)
        nc.sync.dma_start(out=sel2_sb[:], in_=sel2)

        m_sb = const.tile([128, KT2, QS], f32)
        s_sb = {h: spool.tile([128, KT2, QS], bf16, tag="s", name=f"s{h}")
                for h in exp_heads}
        flat = [const.tile([128, QS], bf16, tag=f"flat{j}", name=f"flat{j}")
                for j in range(4)]
        norms = [const.tile([2, QS], f32, tag=f"norms{j}", name=f"norms{j}")
                 for j in range(4)]

        # ---- Phase A: distance matmuls; evacuate -d2 to fp32 SBUF m ----
        for kt in range(KT2):
            d2 = psp.tile([128, QS], f32, tag="ps", name=f"d2_{kt}")
            for qc in range(2):
                s5 = slice(qc * 512, (qc + 1) * 512)
                nc.tensor.matmul(d2[:, s5],
                                 lhsT=ka_sb[:, kt * 128:(kt + 1) * 128],
                                 rhs=qa_sb[:, s5], start=True, stop=True)
            nc.vector.tensor_copy(out=m_sb[:, kt, :], in_=d2[:])

        # ---- ACT exp streams (per k-tile so attends can chase) ----
        for h in exp_heads:
            for kt in range(KT2):
                nc.scalar.activation(out=s_sb[h][:, kt, :], in_=m_sb[:, kt, :],
                                     func=AF.Exp, scale=float(cv[h]))

        # evac helper: att psum -> flat rows + normalizer row -> norms[h]
        # (PSUM reads are DVE-only: GPSIMD cannot access PSUM, ACT is busy
        # with the exp streams)
        def evac(att, h):
            j, r0 = h // 2, (h % 2) * 64
            nc.vector.tensor_copy(out=flat[j][r0:r0 + 64, :], in_=att[0:64, :])
            stg = stage.tile([V1, QS], f32, tag="stg", name=f"stg{h}")
            nc.vector.tensor_copy(out=stg[64:65, :], in_=att[64:65, :])
            nc.sync.dma_start(out=norms[j][h % 2:h % 2 + 1, :],
                              in_=stg[64:65, :])

        def attend(h):
            att = psp.tile([V1, QS], f32, tag="ps", name=f"att{h}")
            for qc in range(2):
                s5 = slice(qc * 512, (qc + 1) * 512)
                for kt in range(KT2):
                    nc.tensor.matmul(att[:, s5],
                                     lhsT=vp_sb[:, kt, h * V1:(h + 1) * V1],
                                     rhs=s_sb[h][:, kt, s5],
                                     start=(kt == 0), stop=(kt == KT2 - 1))
            evac(att, h)

        # pair normalization: r = 1/(n+eps) on DVE, partition-broadcast of
        # the head-pair reciprocals via a K=4 hi/lo selection matmul.
        done_heads = set()
        pair_done = set()

        def mark(h):
            done_heads.add(h)
            j = h // 2
            if not (2 * j in done_heads and 2 * j + 1 in done_heads):
                return
            ne = rpool.tile([2, QS], f32, tag="ne", name=f"ne{j}")
            nc.vector.tensor_scalar_add(ne[:], norms[j][:], 1e-5)
            rj = rpool.tile([2, QS], f32, tag="rj", name=f"rj{j}")
            nc.vector.reciprocal(rj[:], ne[:])
            rhi = rpool.tile([2, QS], bf16, tag="rhi", name=f"rhi{j}")
            nc.vector.tensor_copy(out=rhi[:], in_=rj[:])
            rlo = rpool.tile([2, QS], bf16, tag="rlo", name=f"rlo{j}")
            nc.vector.tensor_sub(rlo[:], rj[:], rhi[:])
            rb = psp.tile([128, QS], f32, tag="ps", name=f"rb{j}")
            for qc in range(2):
                s5 = slice(qc * 512, (qc + 1) * 512)
                nc.tensor.matmul(rb[:, s5], lhsT=sel2_sb[:],
                                 rhs=rhi[:, s5], start=True, stop=False)
                nc.tensor.matmul(rb[:, s5], lhsT=sel2_sb[:],
                                 rhs=rlo[:, s5], start=False, stop=True)
            nc.vector.tensor_mul(flat[j][:], flat[j][:], rb[:])
            pair_done.add(j)

        # ---- PE schedule: att[first], poly block, remaining attends ----
        attend(exp_heads[0])
        mark(exp_heads[0])
        for i, h in enumerate(poly_list):
            G = psp.tile([R, V1], f32, tag="ps", name=f"G{h}")
            for kt in range(KT2):
                nc.tensor.matmul(G[:],
                                 lhsT=wp_sb[:, kt, i * R:(i + 1) * R],
                                 rhs=vp_sb[:, kt, h * V1:(h + 1) * V1],
                                 start=(kt == 0), stop=(kt == KT2 - 1))
            g_sb = gpool.tile([R, V1], bf16, tag="g", name=f"g{h}")
            nc.vector.tensor_copy(out=g_sb[:], in_=G[:])
            attp = psp.tile([V1, QS], f32, tag="ps", name=f"attp{h}")
            for qc in range(2):
                s5 = slice(qc * 512, (qc + 1) * 512)
                nc.tensor.matmul(attp[:, s5], lhsT=g_sb[:],
                                 rhs=ut_sb[:, s5], start=True, stop=True)
            evac(attp, h)
            mark(h)
        for h in exp_heads[1:]:
            attend(h)
            mark(h)
        assert len(pair_done) == 4, pair_done

        # ---- out projection: outT[o, q] = sum_hv wt[hv, o] * flat[hv, q] ----
        ev_engs = [nc.scalar, nc.vector, nc.scalar, nc.vector]
        for ot in range(4):
            po = psp.tile([128, QS], f32, tag="ps", name=f"po{ot}")
            for jj, j in enumerate([3, 2, 1, 0]):
                for qc in range(2):
                    s5 = slice(qc * 512, (qc + 1) * 512)
                    nc.tensor.matmul(po[:, s5],
                                     lhsT=wt_sb[:, j, ot * 128:(ot + 1) * 128],
                                     rhs=flat[j][:, s5],
                                     start=(jj == 0), stop=(jj == 3))
            ob = obuf.tile([128, QS], f32, tag="ob", name=f"ob{ot}")
            e = ev_engs[ot]
            if e is nc.scalar:
                e.copy(out=ob[:], in_=po[:])
            else:
                e.tensor_copy(out=ob[:], in_=po[:])
            nc.sync.dma_start(out=outT[ot * 128:(ot + 1) * 128, :], in_=ob[:])

    nc.compile()
    _cache[key] = nc
    return nc


def _hilo(x, bf16):
    hi = x.astype(bf16)
    lo = (x - hi.astype(np.float32)).astype(bf16)
    return hi, lo


def _build_U(q):
    q2s = (q ** 2).sum(-1)
    return np.stack([(q2s ** a) * (q[:, 0] ** c1) * (q[:, 1] ** c2)
                     * (q[:, 2] ** c3) for a, c1, c2, c3 in FEATS], 1)


def _build_W(k, coeffs, deg):
    k2s = (k ** 2).sum(-1)
    cols = []
    for a, c1, c2, c3 in FEATS:
        cc = c1 + c2 + c3
        col = np.zeros(len(k))
        for j in range(a + cc, deg + 1):
            bb = j - a - cc
            mult = factorial(j) / (factorial(a) * factorial(bb)
                                   * factorial(c1) * factorial(c2)
                                   * factorial(c3))
            col += coeffs[j] * mult * ((-2.0) ** cc) * (k2s ** bb) \
                * (k[:, 0] ** c1) * (k[:, 1] ** c2) * (k[:, 2] ** c3)
        cols.append(col)
    return np.stack(cols, 1)


def _prep_batch(kp_b, vals_b, mask_b, KT2, poly, bf16):
    """Per-batch key-side prep: compact unmasked keys, pad to KT2*128."""
    NK = KT2 * 128
    idx = np.where(~mask_b)[0]
    nk = len(idx)
    kpos = np.zeros((NK, DPOS), np.float32)
    kpos[:nk] = kp_b[idx]
    k2 = (kpos * kpos).sum(-1)
    ones_pad = np.zeros(NK, np.float32)
    ones_pad[:nk] = 1.0
    ka5 = np.stack([kpos[:, 0], kpos[:, 1], kpos[:, 2], k2, ones_pad])
    ka_hi, ka_lo = _hilo(ka5.astype(np.float32), bf16)
    ka = np.concatenate([ka_hi, ka_lo, ka_hi])          # [15, NK]
    vv = np.zeros((NK, H, V1), np.float32)
    vv[:nk, :, :V] = vals_b[idx]
    vv[:nk, :, V] = 1.0
    vp = vv.reshape(KT2, 128, H * V1).transpose(1, 0, 2).astype(bf16)
    wp = None
    if poly:
        wcols = []
        for h in sorted(poly):
            deg, coeffs = poly[h]
            W = _build_W(kpos.astype(np.float64), coeffs, deg)
            W[nk:] = 0.0
            wcols.append(W.astype(np.float32))
        Wall = np.concatenate(wcols, 1)                 # [NK, n_poly*R]
        wp = Wall.reshape(KT2, 128, -1).transpose(1, 0, 2).astype(bf16)
    return {"ka": np.ascontiguousarray(ka),
            "vp": np.ascontiguousarray(vp),
            "wp": np.ascontiguousarray(wp) if wp is not None else None}


def _prep_core(qp_half, poly, bf16):
    q2 = (qp_half * qp_half).sum(-1)
    one_q = np.ones(QS, np.float32)
    qa5 = np.stack([2 * qp_half[:, 0], 2 * qp_half[:, 1], 2 * qp_half[:, 2],
                    -one_q, -q2]).astype(np.float32)
    qa_hi, qa_lo = _hilo(qa5, bf16)
    qa = np.concatenate([qa_hi, qa_hi, qa_lo])          # [15, QS]
    ut = None
    if poly:
        U = _build_U(qp_half.astype(np.float64))        # [QS, R]
        ut = np.ascontiguousarray(U.T.astype(np.float32)).astype(bf16)
    return {"qa": np.ascontiguousarray(qa), "ut": ut}


def kernel(query_positions, key_positions, values, masked_elements,
           lengthscales, w_out, _want_trace=False):
    import ml_dtypes
    from concourse.bass_utils import run_bass_kernel_spmd

    bf16 = ml_dtypes.bfloat16
    qp = np.asarray(query_positions, np.float32)
    kp = np.asarray(key_positions, np.float32)
    vals = np.asarray(values, np.float32)
    mask = np.asarray(masked_elements).astype(bool)
    ls = np.asarray(lengthscales, np.float32)
    w = np.asarray(w_out, np.float32)

    cv = (1.0 / (ls.astype(np.float64) ** 2)).astype(np.float32)
    maxcnt = int((~mask).sum(1).max())
    KT2 = max(1, -(-maxcnt // 128))
    nc = _build(tuple(float(x) for x in cv), KT2)
    poly, _ = _classify(cv)

    wt = np.ascontiguousarray(w.T).reshape(4, 128, OUTD) \
        .transpose(1, 0, 2).astype(bf16)
    sel2 = np.zeros((2, 128), np.float32)
    sel2[0, :64] = 1.0
    sel2[1, 64:] = 1.0
    sel2 = sel2.astype(bf16)

    bprep = [_prep_batch(kp[b], vals[b], mask[b], KT2, poly, bf16)
             for b in range(B)]
    in_maps = []
    for c in range(NCORES):
        b, hf = c // 2, c % 2
        cprep = _prep_core(qp[b, hf * QS:(hf + 1) * QS], poly, bf16)
        m = {"ka": bprep[b]["ka"], "qa": cprep["qa"], "vp": bprep[b]["vp"],
             "wt": wt, "sel2": sel2}
        if poly:
            m["wp"] = bprep[b]["wp"]
            m["ut"] = cprep["ut"]
        in_maps.append(m)
    res = run_bass_kernel_spmd(nc, in_maps, core_ids=list(range(NCORES)),
                               trace=_want_trace)
    out = np.empty((B, LQ, OUTD), np.float32)
    for c in range(NCORES):
        b, hf = c // 2, c % 2
        out[b, hf * QS:(hf + 1) * QS, :] = res.results[c]["outT"].T
    if _want_trace:
        return out, res
    return out


# revision 30
# speedup vs baseline: 1.4595x; 1.2931x over previous
"""Trainium2 Bass kernel for KernelAttention (gaussian-kernel multi-head attention).

Math (per batch b):
  d2[q,k]   = |q_pos[q] - k_pos[k]|^2   (as -d2 via one K=15 augmented matmul)
  s_h[k,q]  = exp(-c_h * d2),  c_h = 1/lengthscale_h^2
  att_h[q,v]= sum_k s_h[k,q] * V[k,h,v] / (sum_k s_h[k,q]*unmasked[k] + 1e-5)
  out[q,o]  = sum_{h,v} att_h[q,v] * w_out[o, h*64+v]

Sharding: 8 cores = (batch b in 0..3) x (query half in 0..1); each core owns
[1024 q, all keys].  All inputs host-prepped per core; outputs host-gathered.

Key structure (vs. a dense implementation):
- Masked keys are compacted away on the host (~half the keys), shrinking the
  k extent from 16 tiles to KT2 (9 here) tiles.  Padded tail keys carry
  zeroed values/ones-column so they contribute nothing.
- Scores stay transposed [k, q] so the attend matmul (lhsT = values+ones col,
  rhs = scores) contracts k on the PE partition dim; psum row 64 accumulates
  the normalizer via the mask-zeroed ones column.
- m = -d2 is evacuated from PSUM to fp32 SBUF (DVE/Pool alternating), so
  every exp reads full-precision distances (the bf16-m rounding amplified by
  c_h=100 was the baseline's dominant error).
- Large-lengthscale heads (c_h*d2max small) are factored through a rank-70
  polynomial basis: s_h ~= U[q,:] @ W_h[k,:]^T with monomial features of the
  augmented position vectors (host-computed).  Their attend collapses to
  G_h = W_h^T V_h (tiny) and att_h = G_h^T @ U^T, skipping both the exp and
  the O(k*q) attend matmuls.
- Normalization is deferred past the attend; reciprocals on DVE per head
  pair, broadcast across partitions with a K=4 hi/lo selection matmul.
"""

import numpy as np
import itertools
from math import factorial
from contextlib import ExitStack

B, LQ, LK, DPOS = 4, 2048, 2048, 3
H, V, OUTD = 8, 64, 512
QS = LQ // 2          # q rows per core
V1 = V + 1            # value cols + ones col
NCORES = 8
DMAX = 64.0           # poly fit domain [0, DMAX] for d2
PMAX = 4
FEATS = [f for f in itertools.product(range(PMAX + 1), repeat=4)
         if sum(f) <= PMAX]
R = len(FEATS)        # 70 poly features

_cache = {}


def _cheb_power_coeffs(c, deg, tmax):
    """exp(-c*t) ~= sum_j bt[j] t^j on [0, tmax]; returns (bt, max_err)."""
    from numpy.polynomial import chebyshev as C, polynomial as P
    t = (np.cos(np.pi * (np.arange(4000) + 0.5) / 4000) + 1) / 2 * tmax
    f = np.exp(-np.float64(c) * t)
    ch = C.chebfit(t / tmax * 2 - 1, f, deg)
    bpow = C.cheb2poly(ch)
    bt = np.zeros(deg + 1)
    acc = np.array([1.0])
    lin = np.array([-1.0, 2.0 / tmax])
    for j in range(deg + 1):
        bt[:len(acc)] += bpow[j] * acc
        acc = P.polymul(acc, lin)
    err = np.abs(np.polyval(bt[::-1], t) - f).max()
    return bt, err


def _classify(cv):
    """Split heads into poly-factorizable (large lengthscale) and exp heads."""
    poly = {}
    for h in range(H):
        for deg in range(2, PMAX + 1):
            bt, err = _cheb_power_coeffs(cv[h], deg, DMAX)
            if err < 2e-4:
                poly[h] = (deg, bt)
                break
    exp_heads = [h for h in range(H) if h not in poly]
    return poly, exp_heads


def _build(cv, KT2):
    key = (tuple(cv), KT2)
    if key in _cache:
        return _cache[key]
    import concourse.bacc as bacc
    import concourse.tile as tile
    from concourse import mybir

    f32 = mybir.dt.float32
    bf16 = mybir.dt.bfloat16
    AF = mybir.ActivationFunctionType

    poly, exp_heads = _classify(cv)
    n_poly = len(poly)
    poly_list = sorted(poly)
    NK = KT2 * 128

    nc = bacc.Bacc("TRN2", target_bir_lowering=False, debug=False,
                   num_devices=NCORES)
    # ka/qa: hi/lo bf16 split of the K=5 augmented distance operands:
    # rows [hi(5); lo(5); hi(5)] x [hi(5); hi(5); lo(5)] accumulate
    # hi*hi + lo*hi + hi*lo in fp32 PSUM in one K=15 matmul.
    ka = nc.dram_tensor("ka", [15, NK], bf16, kind="ExternalInput").ap()
    qa = nc.dram_tensor("qa", [15, QS], bf16, kind="ExternalInput").ap()
    vp = nc.dram_tensor("vp", [128, KT2, H * V1], bf16, kind="ExternalInput").ap()
    wt = nc.dram_tensor("wt", [128, 4, OUTD], bf16, kind="ExternalInput").ap()
    if n_poly:
        ut = nc.dram_tensor("ut", [R, QS], bf16, kind="ExternalInput").ap()
        wp = nc.dram_tensor("wp", [128, KT2, n_poly * R], bf16,
                            kind="ExternalInput").ap()
    outT = nc.dram_tensor("outT", [OUTD, QS], f32, kind="ExternalOutput").ap()

    with tile.TileContext(nc) as tc, ExitStack() as ctx:
        const = ctx.enter_context(tc.tile_pool(name="const", bufs=1))
        # 4 bufs for 5 score tensors: the 5th head's exp stream starts well
        # after the 1st head's attend has been evacuated, so it reuses buf 0
        spool = ctx.enter_context(tc.tile_pool(name="spool", bufs=4))
        gpool = ctx.enter_context(tc.tile_pool(name="gpool", bufs=2))
        stage = ctx.enter_context(tc.tile_pool(name="stage", bufs=2))
        rpool = ctx.enter_context(tc.tile_pool(name="rpool", bufs=2))
        obuf = ctx.enter_context(tc.tile_pool(name="obuf", bufs=2))
        psp = ctx.enter_context(tc.tile_pool(name="psum", bufs=4, space="PSUM"))

        ka_sb = const.tile([15, NK], bf16)
        nc.sync.dma_start(out=ka_sb[:], in_=ka)
        qa_sb = const.tile([15, QS], bf16)
        nc.sync.dma_start(out=qa_sb[:], in_=qa)
        vp_sb = const.tile([128, KT2, H * V1], bf16)
        nc.sync.dma_start(out=vp_sb[:], in_=vp)
        if n_poly:
            wp_sb = const.tile([128, KT2, n_poly * R], bf16)
            nc.sync.dma_start(out=wp_sb[:], in_=wp)
            ut_sb = const.tile([R, QS], bf16)
            nc.sync.dma_start(out=ut_sb[:], in_=ut)
        wt_sb = const.tile([128, 4, OUTD], bf16)
        nc.sync.dma_start(out=wt_sb[:], in_=wt)
        m_sb = const.tile([128, KT2, QS], f32)
        s_sb = {h: spool.tile([128, KT2, QS], bf16, tag="s", name=f"s{h}")
                for h in exp_heads}
        flat = [const.tile([128, QS], bf16, tag=f"flat{j}", name=f"flat{j}")
                for j in range(4)]
        norms_t = {}

        # ---- Phase A: distance matmuls; evacuate -d2 to fp32 SBUF m ----
        for kt in range(KT2):
            d2 = psp.tile([128, QS], f32, tag="ps", name=f"d2_{kt}")
            for qc in range(2):
                s5 = slice(qc * 512, (qc + 1) * 512)
                nc.tensor.matmul(d2[:, s5],
                                 lhsT=ka_sb[:, kt * 128:(kt + 1) * 128],
                                 rhs=qa_sb[:, s5], start=True, stop=True)
            nc.vector.tensor_copy(out=m_sb[:, kt, :], in_=d2[:])

        # ---- ACT exp streams (per k-tile so attends can chase) ----
        for h in exp_heads:
            for kt in range(KT2):
                nc.scalar.activation(out=s_sb[h][:, kt, :], in_=m_sb[:, kt, :],
                                     func=AF.Exp, scale=float(cv[h]))

        # evac helper: att psum -> flat rows + normalizer row -> norms[h]
        # (PSUM reads are DVE-only: GPSIMD cannot access PSUM, ACT is busy
        # with the exp streams)
        def evac(att, h):
            j, r0 = h // 2, (h % 2) * 64
            nc.vector.tensor_copy(out=flat[j][r0:r0 + 64, :], in_=att[0:64, :])
            stg = stage.tile([V1, QS], f32, tag="stg", name=f"stg{h}")
            nc.vector.tensor_copy(out=stg[64:65, :], in_=att[64:65, :])
            nt = rpool.tile([1, QS], f32, tag="nrm", name=f"nrm{h}", bufs=3)
            norms_t[h] = nt
            # issue from the idle gpsimd DGE so it doesn't queue behind the
            # sync-queue input DMAs
            nc.gpsimd.dma_start(out=nt[:], in_=stg[64:65, :])

        def attend(h):
            att = psp.tile([V1, QS], f32, tag="ps", name=f"att{h}")
            for qc in range(2):
                s5 = slice(qc * 512, (qc + 1) * 512)
                for kt in range(KT2):
                    nc.tensor.matmul(att[:, s5],
                                     lhsT=vp_sb[:, kt, h * V1:(h + 1) * V1],
                                     rhs=s_sb[h][:, kt, s5],
                                     start=(kt == 0), stop=(kt == KT2 - 1))
            evac(att, h)

        # pair normalization: r = 1/(n+eps) on DVE (fast-approx reciprocal,
        # ~18 bits); broadcast across partitions on the otherwise-idle
        # GPSIMD engine (SBUF->SBUF partition_broadcast, exact fp32), so the
        # PE never sees normalization work.
        done_heads = set()
        pair_done = set()
        r_t = {}

        def mark(h):
            done_heads.add(h)
            nt = norms_t[h]
            nc.vector.tensor_scalar_add(nt[:], nt[:], 1e-5)
            rh = rpool.tile([1, QS], f32, tag="rh", name=f"rh{h}", bufs=3)
            nc.vector.reciprocal_approx_fast(out=rh[:], in_=nt[:])
            r_t[h] = rh
            j = h // 2
            if not (2 * j in done_heads and 2 * j + 1 in done_heads):
                return
            # partition_broadcast output must start at partition 0, so
            # broadcast each head's r to a full tile and multiply halves
            # with matching partition ranges.
            rba = rpool.tile([128, QS], f32, tag="rb", name=f"rba{j}")
            nc.gpsimd.partition_broadcast(rba[:], r_t[2 * j][:], channels=128)
            rbb = rpool.tile([128, QS], f32, tag="rb", name=f"rbb{j}")
            nc.gpsimd.partition_broadcast(rbb[:], r_t[2 * j + 1][:],
                                          channels=128)
            nc.vector.tensor_mul(flat[j][0:64, :], flat[j][0:64, :],
                                 rba[0:64, :])
            nc.vector.tensor_mul(flat[j][64:128, :], flat[j][64:128, :],
                                 rbb[64:128, :])
            pair_done.add(j)

        # ---- PE schedule: att[first], poly block, remaining attends ----
        attend(exp_heads[0])
        mark(exp_heads[0])
        for i, h in enumerate(poly_list):
            G = psp.tile([R, V1], f32, tag="ps", name=f"G{h}")
            for kt in range(KT2):
                nc.tensor.matmul(G[:],
                                 lhsT=wp_sb[:, kt, i * R:(i + 1) * R],
                                 rhs=vp_sb[:, kt, h * V1:(h + 1) * V1],
                                 start=(kt == 0), stop=(kt == KT2 - 1))
            g_sb = gpool.tile([R, V1], bf16, tag="g", name=f"g{h}")
            nc.vector.tensor_copy(out=g_sb[:], in_=G[:])
            attp = psp.tile([V1, QS], f32, tag="ps", name=f"attp{h}")
            for qc in range(2):
                s5 = slice(qc * 512, (qc + 1) * 512)
                nc.tensor.matmul(attp[:, s5], lhsT=g_sb[:],
                                 rhs=ut_sb[:, s5], start=True, stop=True)
            evac(attp, h)
            mark(h)
        # ---- out projection: outT[o, q] = sum_hv wt[hv, o] * flat[hv, q] ----
        # po[ot] accumulates j-chunks in order [j3, j0, j1, j2]; the last
        # attend's pair (j2) comes last so earlier chunks overlap the attends.
        po_t = {}

        def po_chunk(ot, j, start=False, stop=False):
            po = po_t.get(ot)
            if po is None:
                po = po_t[ot] = psp.tile([128, QS], f32, tag="ps",
                                         name=f"po{ot}", uniquify=True)
            for qc in range(2):
                s5 = slice(qc * 512, (qc + 1) * 512)
                nc.tensor.matmul(po[:, s5],
                                 lhsT=wt_sb[:, j, ot * 128:(ot + 1) * 128],
                                 rhs=flat[j][:, s5],
                                 start=start, stop=stop)

        def po_evac(ot, eng):
            ob = obuf.tile([128, QS], f32, tag="ob", name=f"ob{ot}")
            if eng is nc.scalar:
                eng.copy(out=ob[:], in_=po_t[ot][:])
            else:
                eng.tensor_copy(out=ob[:], in_=po_t[ot][:])
            nc.sync.dma_start(out=outT[ot * 128:(ot + 1) * 128, :], in_=ob[:])

        last = exp_heads[-1]
        last_j = last // 2
        early_js = [j for j in [3, 0, 1, 2] if j != last_j]
        for h in exp_heads[1:-1]:
            attend(h)
            mark(h)
        # early chunks of po0/po1 hide the last attend's exp-stream lag
        for ot in (0, 1):
            for jj, j in enumerate(early_js[:2]):
                po_chunk(ot, j, start=(jj == 0))
        attend(last)
        mark(last)
        assert len(pair_done) == 4, pair_done
        for ot in (0, 1):
            po_chunk(ot, early_js[2])
            po_chunk(ot, last_j, stop=True)
        po_evac(0, nc.scalar)
        po_evac(1, nc.vector)
        for ot in (2, 3):
            for jj, j in enumerate(early_js):
                po_chunk(ot, j, start=(jj == 0))
            po_chunk(ot, last_j, stop=True)
            po_evac(ot, nc.scalar if ot == 2 else nc.vector)

    nc.compile()
    _cache[key] = nc
    return nc


def _hilo(x, bf16):
    hi = x.astype(bf16)
    lo = (x - hi.astype(np.float32)).astype(bf16)
    return hi, lo


def _build_U(q):
    q2s = (q ** 2).sum(-1)
    return np.stack([(q2s ** a) * (q[:, 0] ** c1) * (q[:, 1] ** c2)
                     * (q[:, 2] ** c3) for a, c1, c2, c3 in FEATS], 1)


def _build_W(k, coeffs, deg):
    k2s = (k ** 2).sum(-1)
    cols = []
    for a, c1, c2, c3 in FEATS:
        cc = c1 + c2 + c3
        col = np.zeros(len(k))
        for j in range(a + cc, deg + 1):
            bb = j - a - cc
            mult = factorial(j) / (factorial(a) * factorial(bb)
                                   * factorial(c1) * factorial(c2)
                                   * factorial(c3))
            col += coeffs[j] * mult * ((-2.0) ** cc) * (k2s ** bb) \
                * (k[:, 0] ** c1) * (k[:, 1] ** c2) * (k[:, 2] ** c3)
        cols.append(col)
    return np.stack(cols, 1)


def _prep_batch(kp_b, vals_b, mask_b, KT2, poly, bf16):
    """Per-batch key-side prep: compact unmasked keys, pad to KT2*128."""
    NK = KT2 * 128
    idx = np.where(~mask_b)[0]
    nk = len(idx)
    kpos = np.zeros((NK, DPOS), np.float32)
    kpos[:nk] = kp_b[idx]
    k2 = (kpos * kpos).sum(-1)
    ones_pad = np.zeros(NK, np.float32)
    ones_pad[:nk] = 1.0
    ka5 = np.stack([kpos[:, 0], kpos[:, 1], kpos[:, 2], k2, ones_pad])
    ka_hi, ka_lo = _hilo(ka5.astype(np.float32), bf16)
    ka = np.concatenate([ka_hi, ka_lo, ka_hi])          # [15, NK]
    vv = np.zeros((NK, H, V1), np.float32)
    vv[:nk, :, :V] = vals_b[idx]
    vv[:nk, :, V] = 1.0
    vp = vv.reshape(KT2, 128, H * V1).transpose(1, 0, 2).astype(bf16)
    wp = None
    if poly:
        wcols = []
        for h in sorted(poly):
            deg, coeffs = poly[h]
            W = _build_W(kpos.astype(np.float64), coeffs, deg)
            W[nk:] = 0.0
            wcols.append(W.astype(np.float32))
        Wall = np.concatenate(wcols, 1)                 # [NK, n_poly*R]
        wp = Wall.reshape(KT2, 128, -1).transpose(1, 0, 2).astype(bf16)
    return {"ka": np.ascontiguousarray(ka),
            "vp": np.ascontiguousarray(vp),
            "wp": np.ascontiguousarray(wp) if wp is not None else None}


def _prep_core(qp_half, poly, bf16):
    q2 = (qp_half * qp_half).sum(-1)
    one_q = np.ones(QS, np.float32)
    qa5 = np.stack([2 * qp_half[:, 0], 2 * qp_half[:, 1], 2 * qp_half[:, 2],
                    -one_q, -q2]).astype(np.float32)
    qa_hi, qa_lo = _hilo(qa5, bf16)
    qa = np.concatenate([qa_hi, qa_hi, qa_lo])          # [15, QS]
    ut = None
    if poly:
        U = _build_U(qp_half.astype(np.float64))        # [QS, R]
        ut = np.ascontiguousarray(U.T.astype(np.float32)).astype(bf16)
    return {"qa": np.ascontiguousarray(qa), "ut": ut}


def kernel(query_positions, key_positions, values, masked_elements,
           lengthscales, w_out, _want_trace=False):
    import ml_dtypes
    from concourse.bass_utils import run_bass_kernel_spmd

    bf16 = ml_dtypes.bfloat16
    qp = np.asarray(query_positions, np.float32)
    kp = np.asarray(key_positions, np.float32)
    vals = np.asarray(values, np.float32)
    mask = np.asarray(masked_elements).astype(bool)
    ls = np.asarray(lengthscales, np.float32)
    w = np.asarray(w_out, np.float32)

    cv = (1.0 / (ls.astype(np.float64) ** 2)).astype(np.float32)
    maxcnt = int((~mask).sum(1).max())
    KT2 = max(1, -(-maxcnt // 128))
    nc = _build(tuple(float(x) for x in cv), KT2)
    poly, _ = _classify(cv)

    wt = np.ascontiguousarray(w.T).reshape(4, 128, OUTD) \
        .transpose(1, 0, 2).astype(bf16)
    bprep = [_prep_batch(kp[b], vals[b], mask[b], KT2, poly, bf16)
             for b in range(B)]
    in_maps = []
    for c in range(NCORES):
        b, hf = c // 2, c % 2
        cprep = _prep_core(qp[b, hf * QS:(hf + 1) * QS], poly, bf16)
        m = {"ka": bprep[b]["ka"], "qa": cprep["qa"], "vp": bprep[b]["vp"],
             "wt": wt}
        if poly:
            m["wp"] = bprep[b]["wp"]
            m["ut"] = cprep["ut"]
        in_maps.append(m)
    res = run_bass_kernel_spmd(nc, in_maps, core_ids=list(range(NCORES)),
                               trace=_want_trace)
    out = np.empty((B, LQ, OUTD), np.float32)
    for c in range(NCORES):
        b, hf = c // 2, c % 2
        out[b, hf * QS:(hf + 1) * QS, :] = res.results[c]["outT"].T
    if _want_trace:
        return out, res
    return out
